# revision 1
# baseline (speedup 1.0000x reference)
"""Trainium2 Bass kernel for nn_Conv1d_NN_spatial (retrieval_knn).

Problem (per batch b, 8 batches -> 8 NeuronCores, data parallel):
  x [64, 4096] queries, y [64, 1024] keys
  dist2[n, m] = ||x_n||^2 + ||y_m||^2 - 2 x_n.y_m ; idx = 3 smallest per n
  out[oc, n] = relu(sum_k W_k @ x[:, idx[n, k]] + b)

Device algorithm (per core):
  key[n, m] = x_n.y_m - 0.5||y_m||^2  (maximize key <=> minimize dist; norm_x
  dropped - constant per row; sqrt dropped - monotone).
  The dot product is computed in 3-limb bf16 arithmetic (xh+xm+xl) so the key
  matches CPU-fp32 precision (~2e-6) at full bf16 PE speed: limb pairs
  (h,l)+(l,h), (m,m)+(h,m), (m,h)+(h,h) as three K=128 matmuls accumulated
  small->large into PSUM, plus a K=4 matmul adding -0.5||y||^2 (4 bf16 limbs).
  Top-3 per row via DVE max8/max_index. Conv reduced to a row gather of
  Zt[m] = [W_0^T x_m | W_1^T x_m | W_2^T x_m] + b/3 (built on device by a tiny
  fp32 matmul, stored k-major [3*1024, 64] in DRAM), 3 indirect-DMA gathers per
  128-row chunk, then a PSUM-accumulated PE transpose sums over k and yields
  [oc, n] directly; ACT applies ReLU.
"""
import os
import sys

sys.path.insert(0, "/opt/trn_rl_repo")

import numpy as np
import ml_dtypes
from contextlib import ExitStack

import concourse.bass as bass
import concourse.tile as tile
from concourse import bacc, mybir
from concourse.bass import IndirectOffsetOnAxis
from concourse.bass_utils import run_bass_kernel_spmd

BF16 = ml_dtypes.bfloat16
B, C, N, M, K, OC = 8, 64, 4096, 1024, 3, 64
P = 128
NCHUNK = N // P  # 32


def _build(reps: int = 1):
    ablate = os.environ.get("ABLATE", "")
    nc = bacc.Bacc("TRN2", target_bir_lowering=False, debug=False, num_devices=8)
    f32, bf16, u32 = mybir.dt.float32, mybir.dt.bfloat16, mybir.dt.uint32
    Relu = mybir.ActivationFunctionType.Relu

    LAd = nc.dram_tensor("la", [P, N], bf16, kind="ExternalInput").ap()
    LBd = nc.dram_tensor("lb", [P, N], bf16, kind="ExternalInput").ap()
    RAd = nc.dram_tensor("ra", [P, M], bf16, kind="ExternalInput").ap()
    RBd = nc.dram_tensor("rb", [P, M], bf16, kind="ExternalInput").ap()
    RCd = nc.dram_tensor("rc", [P, M], bf16, kind="ExternalInput").ap()
    RNd = nc.dram_tensor("rn", [4, M], bf16, kind="ExternalInput").ap()
    ONd = nc.dram_tensor("on", [4, P], bf16, kind="ExternalInput").ap()
    XCd = nc.dram_tensor("xc", [C + 1, M], f32, kind="ExternalInput").ap()
    WTd = nc.dram_tensor("wt", [C + 1, K * OC], f32, kind="ExternalInput").ap()
    IDd = nc.dram_tensor("idy", [P, P], f32, kind="ExternalInput").ap()
    OUTd = nc.dram_tensor("out", [OC, N], f32, kind="ExternalOutput").ap()

    with tile.TileContext(nc) as tc, ExitStack() as ctx:
        cn = ctx.enter_context(tc.tile_pool(name="cn", bufs=1))
        wk = ctx.enter_context(tc.tile_pool(name="wk", bufs=4))
        gk = ctx.enter_context(tc.tile_pool(name="gk", bufs=9))
        zw = ctx.enter_context(tc.tile_pool(name="zw", bufs=2))
        pk = ctx.enter_context(tc.tile_pool(name="pk", bufs=3, space="PSUM"))
        pt = ctx.enter_context(tc.tile_pool(name="pt", bufs=2, space="PSUM"))
        pz = pt  # zt-build psum shares the transpose pool's slots (tag below)
        dr = ctx.enter_context(tc.tile_pool(name="dr", bufs=1, space="DRAM"))

        # constant loads (once)
        XC = cn.tile([C + 1, M], f32)
        nc.sync.dma_start(XC[:], XCd[:])
        WT = cn.tile([C + 1, K * OC], f32)
        nc.sync.dma_start(WT[:], WTd[:])
        RA = cn.tile([P, M], bf16)
        nc.sync.dma_start(RA[:], RAd[:])
        RB = cn.tile([P, M], bf16)
        nc.sync.dma_start(RB[:], RBd[:])
        RC = cn.tile([P, M], bf16)
        nc.sync.dma_start(RC[:], RCd[:])
        RN = cn.tile([4, M], bf16)
        nc.sync.dma_start(RN[:], RNd[:])
        ON = cn.tile([4, P], bf16)
        nc.sync.dma_start(ON[:], ONd[:])
        IDY = cn.tile([P, P], f32)
        nc.sync.dma_start(IDY[:], IDd[:])
        # x limb tensors, split in 4 column blocks for early pipeline start
        LA, LB = [], []
        for t in range(4):
            la = cn.tile([P, N // 4], bf16, tag=f"la{t}")
            nc.sync.dma_start(la[:], LAd[:, t * (N // 4):(t + 1) * (N // 4)])
            LA.append(la)
            lb = cn.tile([P, N // 4], bf16, tag=f"lb{t}")
            nc.sync.dma_start(lb[:], LBd[:, t * (N // 4):(t + 1) * (N // 4)])
            LB.append(lb)
        OUT_SB = cn.tile([OC, N], f32)

        def body(_i=None):
            if ablate == "loadonly":
                nc.scalar.copy(OUT_SB[:, 0:P], IDY[0:OC, :])
                for t in range(4):
                    nc.vector.tensor_copy(
                        OUT_SB[0:P // 2, 2 * P:2 * P + 8].bitcast(bf16),
                        LA[t][0:P // 2, 0:16])
                    nc.vector.tensor_copy(
                        OUT_SB[0:P // 2, 3 * P:3 * P + 8].bitcast(bf16),
                        LB[t][0:P // 2, 0:16])
                nc.vector.tensor_copy(OUT_SB[0:4, 4 * P:4 * P + M // 2].bitcast(bf16), RN[:])
                nc.vector.tensor_copy(OUT_SB[0:P // 2, 5 * P:5 * P + 32].bitcast(bf16), RA[:P // 2, :64])
                nc.vector.tensor_copy(OUT_SB[0:P // 2, 6 * P:6 * P + 32].bitcast(bf16), RB[:P // 2, :64])
                nc.vector.tensor_copy(OUT_SB[0:P // 2, 7 * P:7 * P + 32].bitcast(bf16), RC[:P // 2, :64])
                nc.vector.tensor_copy(OUT_SB[0:4, 8 * P:8 * P + P // 2].bitcast(bf16), ON[:])
                nc.scalar.copy(OUT_SB[0:C + 1, 9 * P:9 * P + M], XC[:])
                nc.scalar.copy(OUT_SB[0:C + 1, 13 * P:13 * P + K * OC], WT[:])
                nc.sync.dma_start(OUTd[:], OUT_SB[:])
                return
            # ---- Zt table build: Zt[m, (k,oc)] = sum_c xc[c, m] wt[c, (k,oc)]
            # stored k-major [3*1024, 64] in DRAM for 256B-row gathers
            ZT = dr.tile([K * M, OC], f32)
            for t in range(M // P):
                zp = pz.tile([P, K * OC], f32, tag="tr", space="PSUM")
                nc.tensor.matmul(
                    zp[:], XC[:, t * P:(t + 1) * P], WT[:], start=True, stop=True
                )
                zs = zw.tile([P, K * OC], f32, tag="zs")
                nc.scalar.copy(zs[:], zp[:])
                for k in range(K):
                    nc.sync.dma_start(
                        ZT[:].rearrange("(k m) o -> k m o", k=K)[k, t * P:(t + 1) * P, :],
                        zs[:, k * OC:(k + 1) * OC],
                    )

            # ---- main chunk loop
            for c in range(NCHUNK):
                la = LA[c // 8][:, (c % 8) * P:(c % 8 + 1) * P]
                lb = LB[c // 8][:, (c % 8) * P:(c % 8 + 1) * P]
                kp = pk.tile([P, M], f32, tag="kp", space="PSUM")
                for h in range(2):
                    hs = slice(h * 512, (h + 1) * 512)
                    nc.tensor.matmul(kp[:, hs], la, RA[:, hs], start=True, stop=False)
                    nc.tensor.matmul(kp[:, hs], lb, RB[:, hs], start=False, stop=False)
                    nc.tensor.matmul(kp[:, hs], lb, RC[:, hs], start=False, stop=False)
                    nc.tensor.matmul(kp[:, hs], ON[:, :P], RN[:, hs], start=False, stop=True)
                ks = wk.tile([P, M], f32, tag="ks")
                nc.scalar.copy(ks[:], kp[:])
                m8 = wk.tile([P, 8], f32, tag="m8")
                i8 = wk.tile([P, 8], u32, tag="i8")
                if ablate == "nomax":
                    nc.vector.tensor_copy(m8[:], ks[:, 0:8])
                    nc.gpsimd.memset(i8[:], 0)
                else:
                    nc.vector.max(m8[:], ks[:])
                    nc.vector.max_index(i8[:], m8[:], ks[:])

                tr = pt.tile([OC, P], f32, tag="tr", space="PSUM")
                for k in range(K):
                    g = gk.tile([P, OC], f32, tag="g")
                    if ablate == "nogather":
                        nc.sync.dma_start(g[:], ZT[:].rearrange(
                            "(k m) o -> k m o", k=K)[k, :P, :])
                    else:
                        nc.gpsimd.indirect_dma_start(
                            out=g[:],
                            out_offset=None,
                            in_=ZT[:],
                            in_offset=IndirectOffsetOnAxis(ap=i8[:, k:k + 1], axis=0),
                            element_offset=k * M * OC,
                        )
                    nc.tensor.matmul(
                        tr[:], g[:], IDY[:], is_transpose=True,
                        start=(k == 0), stop=(k == K - 1),
                    )
                nc.scalar.activation(OUT_SB[:, c * P:(c + 1) * P], tr[:], Relu)
            nc.sync.dma_start(OUTd[:], OUT_SB[:])

        if reps == 1:
            body()
        else:
            with tc.For_i(0, reps, 1) as i:
                body(i)

    nc.compile()
    return nc


_CACHE = {}


def _get_program(reps: int = 1):
    if reps not in _CACHE:
        _CACHE[reps] = _build(reps)
    return _CACHE[reps]


def _limbs(a):
    h = a.astype(BF16).astype(np.float32)
    m = (a - h).astype(BF16).astype(np.float32)
    l = (a - h - m).astype(BF16).astype(np.float32)
    return h, m, l


def prep_core_inputs(xb, yb, conv_w, conv_b):
    """Host-side prep for one batch: limb decomposition + aug tensors."""
    xh, xm, xl = _limbs(xb)
    yh, ym, yl = _limbs(yb)
    la = np.concatenate([xh, xl], 0).astype(BF16)
    lb = np.concatenate([xm, xh], 0).astype(BF16)
    ra = np.concatenate([yl, yh], 0).astype(BF16)
    rb = np.concatenate([ym, ym], 0).astype(BF16)
    rc = np.concatenate([yh, yh], 0).astype(BF16)
    nrm = -0.5 * (yb.astype(np.float64) ** 2).sum(0)
    rn = np.zeros((4, M), BF16)
    r = nrm
    for j in range(4):
        rn[j] = r.astype(BF16)
        r = r - rn[j].astype(np.float64)
    on = np.ones((4, P), BF16)
    xc = np.concatenate([xb[:, :M], np.ones((1, M), np.float32)], 0)
    wt = np.zeros((C + 1, K * OC), np.float32)
    for k in range(K):
        wt[:C, k * OC:(k + 1) * OC] = conv_w[:, :, k].T
        wt[C, k * OC:(k + 1) * OC] = conv_b / K
    idy = np.eye(P, dtype=np.float32)
    return {
        "la": la, "lb": lb, "ra": ra, "rb": rb, "rc": rc, "rn": rn,
        "on": on, "xc": xc, "wt": wt, "idy": idy,
    }


def _in_maps(x, y, conv_w, conv_b):
    return [prep_core_inputs(x[b], y[b], conv_w, conv_b) for b in range(B)]


def kernel(x, y, conv_w, conv_b):
    x = np.asarray(x, dtype=np.float32)
    y = np.asarray(y, dtype=np.float32)
    conv_w = np.asarray(conv_w, dtype=np.float32)
    conv_b = np.asarray(conv_b, dtype=np.float32)
    nc = _get_program(1)
    maps = _in_maps(x, y, conv_w, conv_b)
    res = run_bass_kernel_spmd(nc, maps, list(range(B)))
    return np.stack([res.results[b]["out"] for b in range(B)], 0)


def run_sim(x, y, conv_w, conv_b, core=0):
    """CoreSim single-core run for debugging."""
    from concourse.bass_interp import CoreSim

    nc = _get_program(1)
    maps = _in_maps(np.asarray(x, np.float32), np.asarray(y, np.float32),
                    np.asarray(conv_w, np.float32), np.asarray(conv_b, np.float32))
    sim = CoreSim(nc)
    for name, arr in maps[core].items():
        sim.tensor(name)[:] = arr
    sim.simulate(check_with_hw=False)
    return np.array(sim.tensor("out"))



# revision 11
# speedup vs baseline: 6.7906x; 6.7906x over previous
"""Trainium2 Bass kernel for nn_Conv1d_NN_spatial (retrieval_knn).

Problem (per batch b, 8 batches -> 8 NeuronCores, data parallel):
  x [64, 4096] queries, y [64, 1024] keys
  dist2[n, m] = ||x_n||^2 + ||y_m||^2 - 2 x_n.y_m ; idx = 3 smallest per n
  out[oc, n] = relu(sum_k W_k @ x[:, idx[n, k]] + b)

Device algorithm (per core):
  key[n, m] = x_n.y_m - 0.5||y_m||^2  (maximize key <=> minimize dist; norm_x
  dropped - constant per row; sqrt dropped - monotone).
  The dot product is computed in 3-limb bf16 arithmetic (xh+xm+xl) so the key
  matches CPU-fp32 precision (~2e-6) at full bf16 PE speed: limb pairs
  (h,l)+(l,h), (m,m)+(h,m), (m,h)+(h,h) as three K=128 matmuls accumulated
  small->large into PSUM, plus a K=4 matmul adding -0.5||y||^2 (4 bf16 limbs).
  Top-3 per row via DVE max8/max_index. Conv reduced to a row gather of
  Zt[m] = [W_0^T x_m | W_1^T x_m | W_2^T x_m] + b/3 (built on device by a tiny
  fp32 matmul, stored m-major [1024, 192] in DRAM), 3 indirect-DMA gathers per
  128-row chunk (element_offset selects the k-th 64-col section), then a
  PSUM-accumulated PE transpose sums over k and yields [oc, n] directly; ACT
  applies ReLU.

Schedule: DVE (max8+max_index, 2.25us/chunk) is the saturated bottleneck.
  Input DMAs are split into pieces and issued in priority order across the SP
  and ACT HWDGE queues so the first chunk's operands land ~3us in; chunk 0/1
  front halves (key matmuls + copy + max) are emitted before the ZT build so
  DVE starts early; early-chunk gathers lag until ZT lands (deep i8 ring
  absorbs this); output is stored in 8 column groups as they complete.
"""
import sys

sys.path.insert(0, "/opt/trn_rl_repo")

import numpy as np
import ml_dtypes
from contextlib import ExitStack

import concourse.bass as bass
import concourse.tile as tile
from concourse import bacc, mybir
from concourse.bass import IndirectOffsetOnAxis
from concourse.bass_utils import run_bass_kernel_spmd

BF16 = ml_dtypes.bfloat16
B, C, N, M, K, OC = 8, 64, 4096, 1024, 3, 64
P = 128
NCHUNK = N // P  # 32
NPIECE = 8  # la/lb load pieces, 512 cols each
PCOLS = N // NPIECE


def _build(reps: int = 1):
    nc = bacc.Bacc("TRN2", target_bir_lowering=False, debug=False, num_devices=8)
    f32, bf16, u32 = mybir.dt.float32, mybir.dt.bfloat16, mybir.dt.uint32
    Relu = mybir.ActivationFunctionType.Relu

    LAd = nc.dram_tensor("la", [P, N], bf16, kind="ExternalInput").ap()
    LBd = nc.dram_tensor("lb", [P, N], bf16, kind="ExternalInput").ap()
    RAd = nc.dram_tensor("ra", [P, M], bf16, kind="ExternalInput").ap()
    RBd = nc.dram_tensor("rb", [P, M], bf16, kind="ExternalInput").ap()
    RCd = nc.dram_tensor("rc", [P, M], bf16, kind="ExternalInput").ap()
    RNd = nc.dram_tensor("rn", [4, M], bf16, kind="ExternalInput").ap()
    ONd = nc.dram_tensor("on", [4, P], bf16, kind="ExternalInput").ap()
    XCd = nc.dram_tensor("xc", [C + 1, M], f32, kind="ExternalInput").ap()
    WTd = nc.dram_tensor("wt", [C + 1, K * OC], f32, kind="ExternalInput").ap()
    IDd = nc.dram_tensor("idy", [P, P], f32, kind="ExternalInput").ap()
    OUTd = nc.dram_tensor("out", [OC, N], f32, kind="ExternalOutput").ap()

    with tile.TileContext(nc) as tc, ExitStack() as ctx:
        cn = ctx.enter_context(tc.tile_pool(name="cn", bufs=1))
        wk = ctx.enter_context(tc.tile_pool(name="wk", bufs=4))
        ix = ctx.enter_context(tc.tile_pool(name="ix", bufs=10))
        gk = ctx.enter_context(tc.tile_pool(name="gk", bufs=9))
        zw = ctx.enter_context(tc.tile_pool(name="zw", bufs=2))
        pk = ctx.enter_context(tc.tile_pool(name="pk", bufs=3, space="PSUM"))
        pt = ctx.enter_context(tc.tile_pool(name="pt", bufs=2, space="PSUM"))
        dr = ctx.enter_context(tc.tile_pool(name="dr", bufs=1, space="DRAM"))

        # ---- input loads: priority order, split across SP and ACT queues.
        # chunk-0 critical set first: la piece 0, RA on SP; RB/RC/RN/ON on ACT
        # (behind its fixed LoadActFuncSet); XC/WT for the ZT build + remaining
        # la/lb pieces go on SP so the ACT queue frees up for ks copies.
        LA, LB = [], []
        for t in range(NPIECE):
            la = cn.tile([P, PCOLS], bf16, tag=f"la{t}")
            LA.append(la)
            lb = cn.tile([P, PCOLS], bf16, tag=f"lb{t}")
            LB.append(lb)
        nc.sync.dma_start(LA[0][:], LAd[:, 0:PCOLS])
        RA = cn.tile([P, M], bf16)
        nc.sync.dma_start(RA[:], RAd[:])
        RB = cn.tile([P, M], bf16)
        nc.scalar.dma_start(RB[:], RBd[:])
        nc.sync.dma_start(LB[0][:], LBd[:, 0:PCOLS])
        RC = cn.tile([P, M], bf16)
        nc.scalar.dma_start(RC[:], RCd[:])
        RN = cn.tile([4, M], bf16)
        nc.sync.dma_start(RN[:], RNd[:])
        ON = cn.tile([4, P], bf16)
        nc.scalar.dma_start(ON[:], ONd[:])
        IDY = cn.tile([P, P], f32)
        nc.sync.dma_start(IDY[:], IDd[:])
        XC = cn.tile([C + 1, M], f32)
        nc.sync.dma_start(XC[:], XCd[:])
        WT = cn.tile([C + 1, K * OC], f32)
        nc.sync.dma_start(WT[:], WTd[:])
        for t in range(1, NPIECE):
            nc.sync.dma_start(LA[t][:], LAd[:, t * PCOLS:(t + 1) * PCOLS])
            nc.sync.dma_start(LB[t][:], LBd[:, t * PCOLS:(t + 1) * PCOLS])
        OUT_SB = cn.tile([OC, N], f32)
        # PE pstate warmup fodder: tiny memset tile, matmul'd before real work
        DM = cn.tile([4, 512], bf16, tag="dm")
        nc.gpsimd.memset(DM[:], 0)

        def body(_i=None):
            ZT = dr.tile([M, K * OC], f32)

            # warm the PE clock (p-state ramps only while continuously busy):
            # 3 dummy matmuls bridge the gap until chunk 0's operands land.
            for _ in range(3):
                dmy = pt.tile([P, 512], f32, tag="tr", space="PSUM")
                nc.tensor.matmul(dmy[:], DM[:, 0:P], DM[:], start=True, stop=True)

            def chunk_front(c):
                """key matmuls -> PSUM->SBUF copy -> max8/max_index"""
                la = LA[c // 4][:, (c % 4) * P:(c % 4 + 1) * P]
                lb = LB[c // 4][:, (c % 4) * P:(c % 4 + 1) * P]
                kp = pk.tile([P, M], f32, tag="kp", space="PSUM")
                for h in range(2):
                    hs = slice(h * 512, (h + 1) * 512)
                    nc.tensor.matmul(kp[:, hs], la, RA[:, hs], start=True, stop=False)
                for h in range(2):
                    hs = slice(h * 512, (h + 1) * 512)
                    nc.tensor.matmul(kp[:, hs], lb, RB[:, hs], start=False, stop=False)
                for h in range(2):
                    hs = slice(h * 512, (h + 1) * 512)
                    nc.tensor.matmul(kp[:, hs], lb, RC[:, hs], start=False, stop=False)
                for h in range(2):
                    hs = slice(h * 512, (h + 1) * 512)
                    nc.tensor.matmul(kp[:, hs], ON[:, :P], RN[:, hs], start=False,
                                     stop=True)
                ks = wk.tile([P, M], f32, tag="ks")
                nc.scalar.copy(ks[:], kp[:])
                m8 = wk.tile([P, 8], f32, tag="m8")
                i8 = ix.tile([P, 8], u32, tag="i8")
                nc.vector.max(m8[:], ks[:])
                nc.vector.max_index(i8[:], m8[:], ks[:])
                return i8

            def chunk_back(c, i8):
                """merged indirect gather -> PE transpose-accumulate -> ReLU

                One indirect DMA gathers the full 192-el ZT row of each of the
                3 neighbors (row r gets [row(i0) | row(i1) | row(i2)], 576 els)
                so only one descriptor-gen slice + one DMA semaphore edge per
                chunk; transpose k then reads the diagonal slice k*256..+64
                (neighbor k's section k)."""
                tr = pt.tile([OC, P], f32, tag="tr", space="PSUM")
                for k in range(K):
                    g = gk.tile([P, OC], f32, tag="g")
                    nc.gpsimd.indirect_dma_start(
                        out=g[:],
                        out_offset=None,
                        in_=ZT[:],
                        in_offset=IndirectOffsetOnAxis(ap=i8[:, k:k + 1], axis=0),
                        element_offset=k * OC,
                    )
                    nc.tensor.matmul(
                        tr[:], g[:], IDY[:], is_transpose=True,
                        start=(k == 0), stop=(k == K - 1),
                    )
                nc.scalar.activation(OUT_SB[:, c * P:(c + 1) * P], tr[:], Relu)
                # store finished 4-chunk output groups as they complete
                if c % 4 == 3:
                    g0 = c - 3
                    nc.sync.dma_start(
                        OUTd[:, g0 * P:(c + 1) * P], OUT_SB[:, g0 * P:(c + 1) * P]
                    )

            # ---- Zt table build: Zt[m, (k,oc)] = sum_c xc[c, m] wt[c, (k,oc)]
            # m-major [1024, 192]; row m = [W0^T x_m | W1^T x_m | W2^T x_m]+b/3
            def zt_step(t):
                zp = pt.tile([P, K * OC], f32, tag="tr", space="PSUM")
                nc.tensor.matmul(
                    zp[:], XC[:, t * P:(t + 1) * P], WT[:], start=True, stop=True
                )
                zs = zw.tile([P, K * OC], f32, tag="zs")
                nc.scalar.copy(zs[:], zp[:])
                nc.sync.dma_start(ZT[t * P:(t + 1) * P, :], zs[:])

            # chunk 0-5 fronts first so DVE saturates early, ZT build spread
            # 2 steps/chunk between them (all ZT PE matmuls precede the first
            # transpose matmul - gathers need the full table anyway); then
            # drain the pending backs and run the steady-state loop.
            NPRE = 6
            pend = []
            for c in range(NPRE):
                pend.append(chunk_front(c))
                if 2 <= c < NPRE:
                    zt_step(2 * (c - 2))
                    zt_step(2 * (c - 2) + 1)
            for c in range(NPRE):
                chunk_back(c, pend[c])
            for c in range(NPRE, NCHUNK):
                i8 = chunk_front(c)
                chunk_back(c, i8)

        if reps == 1:
            body()
        else:
            with tc.For_i(0, reps, 1) as i:
                body(i)

    nc.compile()
    return nc


_CACHE = {}


def _get_program(reps: int = 1):
    if reps not in _CACHE:
        _CACHE[reps] = _build(reps)
    return _CACHE[reps]


def _limbs(a):
    h = a.astype(BF16).astype(np.float32)
    m = (a - h).astype(BF16).astype(np.float32)
    l = (a - h - m).astype(BF16).astype(np.float32)
    return h, m, l


def prep_core_inputs(xb, yb, conv_w, conv_b):
    """Host-side prep for one batch: limb decomposition + aug tensors."""
    xh, xm, xl = _limbs(xb)
    yh, ym, yl = _limbs(yb)
    la = np.concatenate([xh, xl], 0).astype(BF16)
    lb = np.concatenate([xm, xh], 0).astype(BF16)
    ra = np.concatenate([yl, yh], 0).astype(BF16)
    rb = np.concatenate([ym, ym], 0).astype(BF16)
    rc = np.concatenate([yh, yh], 0).astype(BF16)
    nrm = -0.5 * (yb.astype(np.float64) ** 2).sum(0)
    rn = np.zeros((4, M), BF16)
    r = nrm
    for j in range(4):
        rn[j] = r.astype(BF16)
        r = r - rn[j].astype(np.float64)
    on = np.ones((4, P), BF16)
    xc = np.concatenate([xb[:, :M], np.ones((1, M), np.float32)], 0)
    wt = np.zeros((C + 1, K * OC), np.float32)
    for k in range(K):
        wt[:C, k * OC:(k + 1) * OC] = conv_w[:, :, k].T
        wt[C, k * OC:(k + 1) * OC] = conv_b / K
    idy = np.eye(P, dtype=np.float32)
    return {
        "la": la, "lb": lb, "ra": ra, "rb": rb, "rc": rc, "rn": rn,
        "on": on, "xc": xc, "wt": wt, "idy": idy,
    }


def _in_maps(x, y, conv_w, conv_b):
    return [prep_core_inputs(x[b], y[b], conv_w, conv_b) for b in range(B)]


def kernel(x, y, conv_w, conv_b):
    x = np.asarray(x, dtype=np.float32)
    y = np.asarray(y, dtype=np.float32)
    conv_w = np.asarray(conv_w, dtype=np.float32)
    conv_b = np.asarray(conv_b, dtype=np.float32)
    nc = _get_program(1)
    maps = _in_maps(x, y, conv_w, conv_b)
    res = run_bass_kernel_spmd(nc, maps, list(range(B)))
    return np.stack([res.results[b]["out"] for b in range(B)], 0)


def run_sim(x, y, conv_w, conv_b, core=0):
    """CoreSim single-core run for debugging."""
    from concourse.bass_interp import CoreSim

    nc = _get_program(1)
    maps = _in_maps(np.asarray(x, np.float32), np.asarray(y, np.float32),
                    np.asarray(conv_w, np.float32), np.asarray(conv_b, np.float32))
    sim = CoreSim(nc)
    for name, arr in maps[core].items():
        sim.tensor(name)[:] = arr
    sim.simulate(check_with_hw=False)
    return np.array(sim.tensor("out"))
